# revision 5
# baseline (speedup 1.0000x reference)
"""Bidirectional 2-layer LSTM (B=32, T=256, IN=H=512) on 8 Trainium2 cores.

2-launch structure (the bidirectional stack forces 512 serial cell steps):
  R0: layer-0 = xg GEMM prologue (xg = x@w_ih + b -> DRAM scratch) followed
      by the 256-step recurrence. fwd on even cores, bwd on odd.
  R1: layer-1, same shape (contraction 1024 in the prologue), consuming h0
      exchanged/reversed between directions by the host.

Recurrence step: four PSUM tiles hold the gates [f,i,g,o] x 32 batch.
xg[t] is seeded into PSUM via identity matmuls (start=True); the 64 w_hh
tile matmuls accumulate on top (start=False), so no elementwise adds are
needed. Activations read PSUM directly with the fp8 descale fused via
`scale=`. w_hh is fp8e4 x64 as the stationary operand only (FWL streams 4
elems/cycle -> short LDWEIGHTS); h stays f16 moving, so only the weights
are quantized (measured end-to-end rel err ~1.3e-2 < 2e-2).
"""

import os
import sys

for _p in ("/opt/trn_rl_repo", "/root/.axon_site/_ro/trn_rl_repo"):
    if os.path.isdir(_p) and _p not in sys.path:
        sys.path.insert(0, _p)

import numpy as np

import concourse.bacc as bacc
import concourse.tile as tile
import concourse.mybir as mybir
import concourse.bass_utils as bass_utils

NCORES = 8
B, T, IN, H = 32, 256, 512, 512
T = int(os.environ.get("LSTM_T", T))  # test override; harness uses 256
NSWEEP = T // 16
TSW = 16
F16 = mybir.dt.float16
F32 = mybir.dt.float32

W_DT = mybir.dt.float8e4   # recurrent-weight dtype (stationary operand only)
WSCALE = 64.0              # lifts w_hh out of fp8 subnormals; descale fused
                           # into the gate activations via scale=1/WSCALE

# gate tile order [f, i, g, o]; OG maps tile-group -> original gate index
# in the reference's [i, f, g, o] column order.
OG = (1, 0, 2, 3)
_PERM = np.concatenate(
    [512 * OG[m // 4] + 128 * (m % 4) + np.arange(128) for m in range(16)]
)

_NC_CACHE = {}


def _build_launch(nkc):
    """One direction-layer: xg GEMM prologue (nkc K-chunks) + T-step LSTM
    recurrence. Inputs (per core):
      rhs   [NSWEEP, nkc, 128, 512] f16 -- moving tiles, cols (t16, b32)
      wih   [nkc, 16, 128, 128] f16     -- input-weight tiles (x WSCALE)
      biasm [16, 128] f32               -- per-(m-tile, partition) bias
      whh   [4, 16, 128, 128] fp8e4     -- recurrent tiles (x WSCALE)
      ident [128, 128] f16              -- identity (PSUM seeding)
    Output:
      hout  [T, 128, 128] f16           -- h_t in [Hw, (hb,b)] layout
    """
    nc = bacc.Bacc("TRN2", target_bir_lowering=False, debug=False,
                   enable_asserts=True, num_devices=NCORES)
    rhs_d = nc.dram_tensor("rhs", [NSWEEP, nkc, 128, 512], F16,
                           kind="ExternalInput")
    wih_d = nc.dram_tensor("wih", [nkc, 16, 128, 128], F16,
                           kind="ExternalInput")
    bias_d = nc.dram_tensor("biasm", [16, 128], F32, kind="ExternalInput")
    whh_d = nc.dram_tensor("whh", [4, 16, 128, 128], W_DT,
                           kind="ExternalInput")
    id_d = nc.dram_tensor("ident", [128, 128], F16, kind="ExternalInput")
    hout_d = nc.dram_tensor("hout", [T, 128, 128], F16,
                            kind="ExternalOutput")
    tick_d = nc.dram_tensor("tick", [128, 1], F32, kind="ExternalOutput")

    inv_s = 1.0 / WSCALE
    SIG = mybir.ActivationFunctionType.Sigmoid
    TANH = mybir.ActivationFunctionType.Tanh

    with tile.TileContext(nc) as tc:
        with (
            tc.tile_pool(name="wpool", bufs=1) as wpool,
            tc.tile_pool(name="dram", bufs=1, space="DRAM") as dram,
            tc.tile_pool(name="rt", bufs=2 * nkc) as rtp,
            tc.tile_pool(name="apsum", bufs=2, space="PSUM") as apsum,
            tc.tile_pool(name="xgsb", bufs=2) as xgp,
            tc.tile_pool(name="xgt", bufs=6) as xgtp,
            tc.tile_pool(name="rpsum", bufs=2, space="PSUM") as rpsum,
            tc.tile_pool(name="state", bufs=1) as statep,
            tc.tile_pool(name="gates", bufs=2) as gatesp,
            tc.tile_pool(name="cell", bufs=2) as cellp,
            tc.tile_pool(name="hbuf", bufs=2) as hbufp,
            tc.tile_pool(name="tmp", bufs=4) as tmpp,
        ):
            # ---- resident weights ----
            wih_sb = wpool.tile([128, nkc * 16 * 128], F16)
            bias_sb = wpool.tile([128, 16], F32)
            whh_sb = wpool.tile([128, 4 * 16 * 128], W_DT)
            id_sb = wpool.tile([128, 128], F16)
            nc.sync.dma_start(
                wih_sb[:].rearrange("k (c m j) -> k c m j",
                                    c=nkc, m=16, j=128),
                wih_d.ap().rearrange("c m k j -> k c m j"))
            nc.sync.dma_start(
                bias_sb[:].rearrange("k m -> k m"),
                bias_d.ap().rearrange("m k -> k m"))
            nc.sync.dma_start(
                whh_sb[:].rearrange("k (c m j) -> k c m j",
                                    c=4, m=16, j=128),
                whh_d.ap().rearrange("c m k j -> k c m j"))
            nc.sync.dma_start(id_sb[:], id_d.ap())

            def wih_t(c, m):
                o = (c * 16 + m) * 128
                return wih_sb[:, o:o + 128]

            def whh_t(c, m):
                o = (c * 16 + m) * 128
                return whh_sb[:, o:o + 128]

            xg_store = dram.tile([T, 128, 512], F16)

            # ---- xg GEMM prologue (PE-dense; keeps HAM warm) ----
            for s in range(NSWEEP):
                rts = []
                for c in range(nkc):
                    rt = rtp.tile([128, 512], F16, name="rt")
                    nc.sync.dma_start(rt[:], rhs_d[s, c])
                    rts.append(rt)
                xg_sb = xgp.tile([128, TSW * 512], F16, name="xg_sb")
                for m in range(16):
                    ps = apsum.tile([128, 512], F32, name="aps", tag="aps")
                    for c in range(nkc):
                        nc.tensor.matmul(ps[:], wih_t(c, m), rts[c][:],
                                         start=(c == 0), stop=(c == nkc - 1))
                    src = ps[:].rearrange("k (t b) -> k t b", t=TSW, b=32)
                    dst = xg_sb[:].rearrange("k (t m b) -> k m t b",
                                             t=TSW, m=16, b=32)[:, m]
                    nc.scalar.activation(
                        dst, src, mybir.ActivationFunctionType.Identity,
                        bias=bias_sb[:, m:m + 1])
                nc.sync.dma_start(
                    xg_store[TSW * s:TSW * (s + 1)].rearrange(
                        "t k c -> k t c"),
                    xg_sb[:].rearrange("k (t c) -> k t c", t=TSW, c=512))

            # ---- recurrence ----
            h_prev = statep.tile([128, 128], F16, tag="h0init")
            c_prev = statep.tile([128, 128], F32, tag="c0init")
            nc.gpsimd.memset(h_prev[:], 0.0)
            nc.gpsimd.memset(c_prev[:], 0.0)

            HB = 8   # h steps per hout DMA (store batching)
            XB = 4   # xg steps per load DMA
            xg4 = None
            hstage = None
            for t in range(T):
                if t % XB == 0:
                    xg4 = xgtp.tile([128, XB * 512], F16, tag="xg4")
                    nc.sync.dma_start(
                        xg4[:].rearrange("k (t c) -> k t c", t=XB, c=512),
                        xg_store[t:t + XB].rearrange("t k c -> k t c"))
                xg_t = xg4[:, 512 * (t % XB):512 * (t % XB + 1)]
                if t % HB == 0:
                    hstage = hbufp.tile([128, HB * 128], F16, tag="hst")
                # gate tile order: f m0-3, i m4-7, g m8-11, o m12-15.
                # Two paired PSUM tiles (i,g) and (f,o) fit beside the
                # prologue's apsum banks. (i,g) is contiguous in xg -> one
                # wide seed; (f,o) needs two seed MMs: the second runs with
                # start=False into the bank's pending-zero region (first
                # write after start zeroes, so it lands exactly).
                ps_ig = rpsum.tile([128, 256], F32, tag="psig")
                ps_fo = rpsum.tile([128, 256], F32, tag="psfo")
                ps_i, ps_g = ps_ig[:, 0:128], ps_ig[:, 128:256]
                ps_f, ps_o = ps_fo[:, 0:128], ps_fo[:, 128:256]
                nc.tensor.matmul(ps_ig[:], id_sb[:], xg_t[:, 128:384],
                                 start=True, stop=False)
                nc.tensor.matmul(ps_f, id_sb[:], xg_t[:, 0:128],
                                 start=True, stop=False)
                nc.tensor.matmul(ps_o, id_sb[:], xg_t[:, 384:512],
                                 start=False, stop=False)
                # section emission order i, g, f, o shortens the serial spine
                secs = [(ps_i, 4, False), (ps_g, 8, True),
                        (ps_f, 0, False), (ps_o, 12, True)]
                for pst, m0, last in secs:
                    for mi in range(4):
                        for c in range(4):
                            nc.tensor.matmul(
                                pst[:, 32 * mi:32 * (mi + 1)],
                                whh_t(c, m0 + mi),
                                h_prev[:, 32 * c:32 * (c + 1)],
                                start=False,
                                stop=(last and mi == 3 and c == 3))
                h_new = hstage[:, 128 * (t % HB):128 * (t % HB + 1)]
                si = gatesp.tile([128, 128], F16, tag="si")
                nc.scalar.activation(si[:], ps_i[:], SIG, scale=inv_s)
                tg = tmpp.tile([128, 128], F16, tag="tg")
                nc.scalar.activation(tg[:], ps_g[:], TANH, scale=inv_s)
                sf = gatesp.tile([128, 128], F16, tag="sf")
                nc.scalar.activation(sf[:], ps_f[:], SIG, scale=inv_s)
                so = tmpp.tile([128, 128], F16, tag="so")
                nc.scalar.activation(so[:], ps_o[:], SIG, scale=inv_s)
                t2 = tmpp.tile([128, 128], F32, tag="t2")
                nc.vector.tensor_mul(t2[:], si[:], tg[:])
                t1 = tmpp.tile([128, 128], F32, tag="t1")
                nc.vector.tensor_mul(t1[:], sf[:], c_prev[:])
                c_new = cellp.tile([128, 128], F32, tag="c")
                nc.vector.tensor_add(c_new[:], t1[:], t2[:])
                th = tmpp.tile([128, 128], F16, tag="th")
                nc.scalar.activation(th[:], c_new[:], TANH)
                nc.vector.tensor_mul(h_new, so[:], th[:])
                c_prev = c_new
                h_prev = h_new
                if t % HB == HB - 1:
                    nc.scalar.dma_start(
                        hout_d[t - HB + 1:t + 1].rearrange("t k c -> k t c"),
                        hstage[:].rearrange("k (t c) -> k t c", t=HB, c=128))
                if t == T - 1:
                    tick = statep.tile([128, 1], F32, tag="tick")
                    nc.vector.tensor_copy(tick[:], h_new[:, 0:1])
                    nc.sync.dma_start(tick_d.ap(), tick[:])

    nc.compile()
    return nc


def _get_nc(nkc):
    if nkc not in _NC_CACHE:
        _NC_CACHE[nkc] = _build_launch(nkc)
    return _NC_CACHE[nkc]


# ---------------- host-side prep ----------------

def _prep_w(w, nkc):
    """[Din, 2048] -> [nkc, 16, 128, 128] f16 tiles, gate-col permuted.
    Scaled by WSCALE so xg arrives pre-scaled for the recurrence (whose
    activations descale by 1/WSCALE)."""
    wp = np.asarray(w, dtype=np.float32)[:, _PERM] * WSCALE
    return np.ascontiguousarray(
        wp.reshape(nkc, 128, 16, 128).transpose(0, 2, 1, 3)).astype(np.float16)


def _prep_whh(w):
    """[512, 2048] -> [4, 16, 128, 128] fp8 tiles (x WSCALE)."""
    wp = np.asarray(w, dtype=np.float32)[:, _PERM] * WSCALE
    tiles = np.ascontiguousarray(
        wp.reshape(4, 128, 16, 128).transpose(0, 2, 1, 3))
    return tiles.astype(mybir.dt.np(W_DT))


def _prep_biasq(b):
    """[2048] -> [16, 128] f32 per-(m-tile, partition) bias, x WSCALE."""
    return np.ascontiguousarray(
        (np.asarray(b, dtype=np.float32) * WSCALE)[_PERM].reshape(16, 128))


def _prep_rhs_from_x(x_dir):
    """[B, T, IN] -> [NSWEEP, 4, 128, 512] f16 with cols (t16, b32)."""
    xt = np.asarray(x_dir, dtype=np.float32).transpose(1, 2, 0)
    r = xt.reshape(NSWEEP, TSW, 4, 128, 32)
    return np.ascontiguousarray(r.transpose(0, 2, 3, 1, 4)).reshape(
        NSWEEP, 4, 128, 512).astype(np.float16)


def _prep_rhs_from_h0(h0):
    """[T, 128, 128] f16 (t, k, 32*hb+b) -> [NSWEEP, 4, 128, 512]."""
    r = h0.reshape(NSWEEP, TSW, 128, 4, 32)
    return np.ascontiguousarray(r.transpose(0, 3, 2, 1, 4)).reshape(
        NSWEEP, 4, 128, 512)


def _unpack_h(h, reverse):
    """[T, 128, 128] f16 -> [B, T, H] f32."""
    a = h.astype(np.float32).reshape(T, 128, 4, 32).transpose(3, 0, 2, 1)
    a = np.ascontiguousarray(a).reshape(B, T, H)
    return a[:, ::-1, :] if reverse else a


_IDENT = np.eye(128, dtype=np.float16)


def _run(nc, in_maps):
    return bass_utils.run_bass_kernel_spmd(
        nc, in_maps, core_ids=list(range(NCORES)), trace=False)


def build_maps_l0(inputs):
    x = np.asarray(inputs["x"], dtype=np.float32)
    fwd = {"rhs": _prep_rhs_from_x(x),
           "wih": _prep_w(inputs["w_ih0f"], 4),
           "biasm": _prep_biasq(inputs["b0f"]),
           "whh": _prep_whh(inputs["w_hh0f"]),
           "ident": _IDENT}
    bwd = {"rhs": _prep_rhs_from_x(x[:, ::-1, :]),
           "wih": _prep_w(inputs["w_ih0b"], 4),
           "biasm": _prep_biasq(inputs["b0b"]),
           "whh": _prep_whh(inputs["w_hh0b"]),
           "ident": _IDENT}
    return [fwd if c % 2 == 0 else bwd for c in range(NCORES)]


def build_maps_l1(inputs, h0f, h0b):
    rhs_f = np.concatenate(
        [_prep_rhs_from_h0(h0f), _prep_rhs_from_h0(h0b[::-1])], axis=1)
    rhs_b = np.concatenate(
        [_prep_rhs_from_h0(h0b), _prep_rhs_from_h0(h0f[::-1])], axis=1)
    wih1f_t = _prep_w(inputs["w_ih1f"], 8)       # chunks 0-3 = h0f half
    wih1b_t = _prep_w(inputs["w_ih1b"], 8)
    wih1b_t = np.concatenate([wih1b_t[4:], wih1b_t[:4]], axis=0)
    fwd = {"rhs": rhs_f, "wih": wih1f_t, "biasm": _prep_biasq(inputs["b1f"]),
           "whh": _prep_whh(inputs["w_hh1f"]), "ident": _IDENT}
    bwd = {"rhs": rhs_b, "wih": wih1b_t, "biasm": _prep_biasq(inputs["b1b"]),
           "whh": _prep_whh(inputs["w_hh1b"]), "ident": _IDENT}
    return [fwd if c % 2 == 0 else bwd for c in range(NCORES)]


def assemble_out(h1f, h1b):
    out = np.concatenate(
        [_unpack_h(h1f, False), _unpack_h(h1b, True)], axis=2)
    return np.ascontiguousarray(out).astype(np.float32)


def kernel(x, w_ih0f, w_hh0f, b0f, w_ih0b, w_hh0b, b0b,
           w_ih1f, w_hh1f, b1f, w_ih1b, w_hh1b, b1b):
    inputs = dict(x=x, w_ih0f=w_ih0f, w_hh0f=w_hh0f, b0f=b0f,
                  w_ih0b=w_ih0b, w_hh0b=w_hh0b, b0b=b0b,
                  w_ih1f=w_ih1f, w_hh1f=w_hh1f, b1f=b1f,
                  w_ih1b=w_ih1b, w_hh1b=w_hh1b, b1b=b1b)

    # ---- launch 1: layer 0 (xg prologue + recurrence) ----
    res = _run(_get_nc(4), build_maps_l0(inputs))
    h0f = res.results[0]["hout"]
    h0b = res.results[1]["hout"]

    # ---- launch 2: layer 1 ----
    res = _run(_get_nc(8), build_maps_l1(inputs, h0f, h0b))
    return assemble_out(res.results[0]["hout"], res.results[1]["hout"])


# revision 7
# speedup vs baseline: 1.2489x; 1.2489x over previous
"""Bidirectional 2-layer LSTM (B=32, T=256, IN=H=512) on 8 Trainium2 cores.

2-launch structure (the bidirectional stack forces 512 serial cell steps):
  R0: layer-0 = xg GEMM prologue (xg = x@w_ih + b -> DRAM scratch) followed
      by the 256-step recurrence. fwd on even cores, bwd on odd.
  R1: layer-1, same shape (contraction 1024 in the prologue), consuming h0
      exchanged/reversed between directions by the host.

Recurrence step: four PSUM tiles hold the gates [f,i,g,o] x 32 batch.
xg[t] is seeded into PSUM via identity matmuls (start=True); the 64 w_hh
tile matmuls accumulate on top (start=False), so no elementwise adds are
needed. Activations read PSUM directly with the fp8 descale fused via
`scale=`. w_hh is fp8e4 x64 as the stationary operand only (FWL streams 4
elems/cycle -> short LDWEIGHTS); h stays f16 moving, so only the weights
are quantized (measured end-to-end rel err ~1.3e-2 < 2e-2).
"""

import os
import sys

for _p in ("/opt/trn_rl_repo", "/root/.axon_site/_ro/trn_rl_repo"):
    if os.path.isdir(_p) and _p not in sys.path:
        sys.path.insert(0, _p)

import numpy as np

import concourse.bacc as bacc
import concourse.tile as tile
import concourse.mybir as mybir
import concourse.bass_utils as bass_utils

NCORES = 8
B, T, IN, H = 32, 256, 512, 512
T = int(os.environ.get("LSTM_T", T))  # test override; harness uses 256
NSWEEP = T // 16
TSW = 16
F16 = mybir.dt.float16
F32 = mybir.dt.float32

W_DT = mybir.dt.float8e4   # recurrent-weight dtype (stationary operand only)
WSCALE = 64.0              # lifts w_hh out of fp8 subnormals; descale fused
                           # into the gate activations via scale=1/WSCALE

# gate tile order [f, i, g, o]; OG maps tile-group -> original gate index
# in the reference's [i, f, g, o] column order.
OG = (1, 0, 2, 3)
_PERM = np.concatenate(
    [512 * OG[m // 4] + 128 * (m % 4) + np.arange(128) for m in range(16)]
)

_NC_CACHE = {}


def _build_launch(nkc):
    """One direction-layer: xg GEMM prologue (nkc K-chunks) + T-step LSTM
    recurrence. Inputs (per core):
      rhs   [NSWEEP, nkc, 128, 512] f16 -- moving tiles, cols (t16, b32)
      wih   [nkc, 16, 128, 128] f16     -- input-weight tiles (x WSCALE)
      biasm [16, 128] f32               -- per-(m-tile, partition) bias
      whh   [4, 16, 128, 128] fp8e4     -- recurrent tiles (x WSCALE)
      ident [128, 128] f16              -- identity (PSUM seeding)
    Output:
      hout  [T, 128, 128] f16           -- h_t in [Hw, (hb,b)] layout
    """
    nc = bacc.Bacc("TRN2", target_bir_lowering=False, debug=False,
                   enable_asserts=True, num_devices=NCORES)
    rhs_d = nc.dram_tensor("rhs", [NSWEEP, nkc, 128, 512], F16,
                           kind="ExternalInput")
    wih_d = nc.dram_tensor("wih", [nkc, 16, 128, 128], F16,
                           kind="ExternalInput")
    bias_d = nc.dram_tensor("biasm", [16, 128], F32, kind="ExternalInput")
    whh_d = nc.dram_tensor("whh", [4, 16, 128, 128], W_DT,
                           kind="ExternalInput")
    id_d = nc.dram_tensor("ident", [128, 128], F16, kind="ExternalInput")
    hout_d = nc.dram_tensor("hout", [T, 128, 128], F16,
                            kind="ExternalOutput")
    tick_d = nc.dram_tensor("tick", [128, 1], F32, kind="ExternalOutput")

    inv_s = 1.0 / WSCALE
    SIG = mybir.ActivationFunctionType.Sigmoid
    TANH = mybir.ActivationFunctionType.Tanh

    with tile.TileContext(nc) as tc:
        with (
            tc.tile_pool(name="wpool", bufs=1) as wpool,
            tc.tile_pool(name="dram", bufs=1, space="DRAM") as dram,
            tc.tile_pool(name="rt", bufs=2 * nkc) as rtp,
            tc.tile_pool(name="apsum", bufs=2, space="PSUM") as apsum,
            tc.tile_pool(name="xgsb", bufs=2) as xgp,
            tc.tile_pool(name="xgt", bufs=6) as xgtp,
            tc.tile_pool(name="rpsum", bufs=2, space="PSUM") as rpsum,
            tc.tile_pool(name="state", bufs=1) as statep,
            tc.tile_pool(name="gates", bufs=2) as gatesp,
            tc.tile_pool(name="cell", bufs=2) as cellp,
            tc.tile_pool(name="hbuf", bufs=2) as hbufp,
            tc.tile_pool(name="tmp", bufs=4) as tmpp,
        ):
            # ---- resident weights ----
            wih_sb = wpool.tile([128, nkc * 16 * 128], F16)
            bias_sb = wpool.tile([128, 16], F32)
            whh_sb = wpool.tile([128, 4 * 16 * 128], W_DT)
            id_sb = wpool.tile([128, 128], F16)
            nc.sync.dma_start(
                wih_sb[:].rearrange("k (c m j) -> k c m j",
                                    c=nkc, m=16, j=128),
                wih_d.ap().rearrange("c m k j -> k c m j"))
            nc.sync.dma_start(
                bias_sb[:].rearrange("k m -> k m"),
                bias_d.ap().rearrange("m k -> k m"))
            nc.sync.dma_start(
                whh_sb[:].rearrange("k (c m j) -> k c m j",
                                    c=4, m=16, j=128),
                whh_d.ap().rearrange("c m k j -> k c m j"))
            nc.sync.dma_start(id_sb[:], id_d.ap())

            def wih_t(c, m):
                o = (c * 16 + m) * 128
                return wih_sb[:, o:o + 128]

            def whh_t(c, m):
                o = (c * 16 + m) * 128
                return whh_sb[:, o:o + 128]

            xg_store = dram.tile([T, 128, 512], F16)

            # ---- xg GEMM, interleaved with the recurrence ----
            # Two sweeps are emitted up front so xg stays 2 sweeps ahead of
            # the recurrence; the remaining units are emitted one per rec
            # step, filling the PE idle left by the step's serial tail.
            sweep_ctx = {}

            def start_sweep(s):
                rts = []
                for c in range(nkc):
                    rt = rtp.tile([128, 512], F16, name="rt")
                    nc.sync.dma_start(rt[:], rhs_d[s, c])
                    rts.append(rt)
                xg_sb = xgp.tile([128, TSW * 512], F16, name="xg_sb")
                sweep_ctx[s] = (rts, xg_sb)

            def emit_unit(s, m):
                rts, xg_sb = sweep_ctx[s]
                ps = apsum.tile([128, 512], F32, name="aps", tag="aps")
                for c in range(nkc):
                    nc.tensor.matmul(ps[:], wih_t(c, m), rts[c][:],
                                     start=(c == 0), stop=(c == nkc - 1))
                src = ps[:].rearrange("k (t b) -> k t b", t=TSW, b=32)
                dst = xg_sb[:].rearrange("k (t m b) -> k m t b",
                                         t=TSW, m=16, b=32)[:, m]
                nc.scalar.activation(
                    dst, src, mybir.ActivationFunctionType.Identity,
                    bias=bias_sb[:, m:m + 1])

            def flush_sweep(s):
                _, xg_sb = sweep_ctx.pop(s)
                nc.sync.dma_start(
                    xg_store[TSW * s:TSW * (s + 1)].rearrange(
                        "t k c -> k t c"),
                    xg_sb[:].rearrange("k (t c) -> k t c", t=TSW, c=512))

            upfront = min(2, NSWEEP)
            for s in range(upfront):
                start_sweep(s)
                for m in range(16):
                    emit_unit(s, m)
                flush_sweep(s)
            units = [(s, m) for s in range(upfront, NSWEEP)
                     for m in range(16)]

            # ---- recurrence ----
            h_prev = statep.tile([128, 128], F16, tag="h0init")
            c_prev = statep.tile([128, 128], F32, tag="c0init")
            nc.gpsimd.memset(h_prev[:], 0.0)
            nc.gpsimd.memset(c_prev[:], 0.0)

            HB = 8   # h steps per hout DMA (store batching)
            XB = 4   # xg steps per load DMA
            xg4 = None
            hstage = None
            for t in range(T):
                if t < len(units):
                    us, um = units[t]
                    if um == 0:
                        start_sweep(us)
                    emit_unit(us, um)
                    if um == 15:
                        flush_sweep(us)
                if t % XB == 0:
                    xg4 = xgtp.tile([128, XB * 512], F16, tag="xg4")
                    nc.sync.dma_start(
                        xg4[:].rearrange("k (t c) -> k t c", t=XB, c=512),
                        xg_store[t:t + XB].rearrange("t k c -> k t c"))
                xg_t = xg4[:, 512 * (t % XB):512 * (t % XB + 1)]
                if t % HB == 0:
                    hstage = hbufp.tile([128, HB * 128], F16, tag="hst")
                # gate tile order: f m0-3, i m4-7, g m8-11, o m12-15.
                # Two paired PSUM tiles (i,g) and (f,o) fit beside the
                # prologue's apsum banks. (i,g) is contiguous in xg -> one
                # wide seed; (f,o) needs two seed MMs: the second runs with
                # start=False into the bank's pending-zero region (first
                # write after start zeroes, so it lands exactly).
                ps_ig = rpsum.tile([128, 256], F32, tag="psig")
                ps_fo = rpsum.tile([128, 256], F32, tag="psfo")
                ps_i, ps_g = ps_ig[:, 0:128], ps_ig[:, 128:256]
                ps_f, ps_o = ps_fo[:, 0:128], ps_fo[:, 128:256]
                nc.tensor.matmul(ps_ig[:], id_sb[:], xg_t[:, 128:384],
                                 start=True, stop=False)
                nc.tensor.matmul(ps_f, id_sb[:], xg_t[:, 0:128],
                                 start=True, stop=False)
                nc.tensor.matmul(ps_o, id_sb[:], xg_t[:, 384:512],
                                 start=False, stop=False)
                # section emission order i, g, f, o shortens the serial spine
                secs = [(ps_i, 4, False), (ps_g, 8, True),
                        (ps_f, 0, False), (ps_o, 12, True)]
                for pst, m0, last in secs:
                    for mi in range(4):
                        for c in range(4):
                            nc.tensor.matmul(
                                pst[:, 32 * mi:32 * (mi + 1)],
                                whh_t(c, m0 + mi),
                                h_prev[:, 32 * c:32 * (c + 1)],
                                start=False,
                                stop=(last and mi == 3 and c == 3))
                h_new = hstage[:, 128 * (t % HB):128 * (t % HB + 1)]
                si = gatesp.tile([128, 128], F16, tag="si")
                nc.scalar.activation(si[:], ps_i[:], SIG, scale=inv_s)
                tg = tmpp.tile([128, 128], F16, tag="tg")
                nc.scalar.activation(tg[:], ps_g[:], TANH, scale=inv_s)
                sf = gatesp.tile([128, 128], F16, tag="sf")
                nc.scalar.activation(sf[:], ps_f[:], SIG, scale=inv_s)
                so = tmpp.tile([128, 128], F16, tag="so")
                nc.scalar.activation(so[:], ps_o[:], SIG, scale=inv_s)
                t2 = tmpp.tile([128, 128], F32, tag="t2")
                nc.vector.tensor_mul(t2[:], si[:], tg[:])
                t1 = tmpp.tile([128, 128], F32, tag="t1")
                nc.vector.tensor_mul(t1[:], sf[:], c_prev[:])
                c_new = cellp.tile([128, 128], F32, tag="c")
                nc.vector.tensor_add(c_new[:], t1[:], t2[:])
                th = tmpp.tile([128, 128], F16, tag="th")
                nc.scalar.activation(th[:], c_new[:], TANH)
                nc.vector.tensor_mul(h_new, so[:], th[:])
                c_prev = c_new
                h_prev = h_new
                if t % HB == HB - 1:
                    nc.scalar.dma_start(
                        hout_d[t - HB + 1:t + 1].rearrange("t k c -> k t c"),
                        hstage[:].rearrange("k (t c) -> k t c", t=HB, c=128))
                if t == T - 1:
                    tick = statep.tile([128, 1], F32, tag="tick")
                    nc.vector.tensor_copy(tick[:], h_new[:, 0:1])
                    nc.sync.dma_start(tick_d.ap(), tick[:])

    nc.compile()
    return nc


def _get_nc(nkc):
    if nkc not in _NC_CACHE:
        _NC_CACHE[nkc] = _build_launch(nkc)
    return _NC_CACHE[nkc]


# ---------------- host-side prep ----------------

def _prep_w(w, nkc):
    """[Din, 2048] -> [nkc, 16, 128, 128] f16 tiles, gate-col permuted.
    Scaled by WSCALE so xg arrives pre-scaled for the recurrence (whose
    activations descale by 1/WSCALE)."""
    wp = np.asarray(w, dtype=np.float32)[:, _PERM] * WSCALE
    return np.ascontiguousarray(
        wp.reshape(nkc, 128, 16, 128).transpose(0, 2, 1, 3)).astype(np.float16)


def _prep_whh(w):
    """[512, 2048] -> [4, 16, 128, 128] fp8 tiles (x WSCALE)."""
    wp = np.asarray(w, dtype=np.float32)[:, _PERM] * WSCALE
    tiles = np.ascontiguousarray(
        wp.reshape(4, 128, 16, 128).transpose(0, 2, 1, 3))
    return tiles.astype(mybir.dt.np(W_DT))


def _prep_biasq(b):
    """[2048] -> [16, 128] f32 per-(m-tile, partition) bias, x WSCALE."""
    return np.ascontiguousarray(
        (np.asarray(b, dtype=np.float32) * WSCALE)[_PERM].reshape(16, 128))


def _prep_rhs_from_x(x_dir):
    """[B, T, IN] -> [NSWEEP, 4, 128, 512] f16 with cols (t16, b32)."""
    xt = np.asarray(x_dir, dtype=np.float32).transpose(1, 2, 0)
    r = xt.reshape(NSWEEP, TSW, 4, 128, 32)
    return np.ascontiguousarray(r.transpose(0, 2, 3, 1, 4)).reshape(
        NSWEEP, 4, 128, 512).astype(np.float16)


def _prep_rhs_from_h0(h0):
    """[T, 128, 128] f16 (t, k, 32*hb+b) -> [NSWEEP, 4, 128, 512]."""
    r = h0.reshape(NSWEEP, TSW, 128, 4, 32)
    return np.ascontiguousarray(r.transpose(0, 3, 2, 1, 4)).reshape(
        NSWEEP, 4, 128, 512)


def _unpack_h(h, reverse):
    """[T, 128, 128] f16 -> [B, T, H] f32."""
    a = h.astype(np.float32).reshape(T, 128, 4, 32).transpose(3, 0, 2, 1)
    a = np.ascontiguousarray(a).reshape(B, T, H)
    return a[:, ::-1, :] if reverse else a


_IDENT = np.eye(128, dtype=np.float16)


def _run(nc, in_maps):
    return bass_utils.run_bass_kernel_spmd(
        nc, in_maps, core_ids=list(range(NCORES)), trace=False)


def build_maps_l0(inputs):
    x = np.asarray(inputs["x"], dtype=np.float32)
    fwd = {"rhs": _prep_rhs_from_x(x),
           "wih": _prep_w(inputs["w_ih0f"], 4),
           "biasm": _prep_biasq(inputs["b0f"]),
           "whh": _prep_whh(inputs["w_hh0f"]),
           "ident": _IDENT}
    bwd = {"rhs": _prep_rhs_from_x(x[:, ::-1, :]),
           "wih": _prep_w(inputs["w_ih0b"], 4),
           "biasm": _prep_biasq(inputs["b0b"]),
           "whh": _prep_whh(inputs["w_hh0b"]),
           "ident": _IDENT}
    return [fwd if c % 2 == 0 else bwd for c in range(NCORES)]


def build_maps_l1(inputs, h0f, h0b):
    rhs_f = np.concatenate(
        [_prep_rhs_from_h0(h0f), _prep_rhs_from_h0(h0b[::-1])], axis=1)
    rhs_b = np.concatenate(
        [_prep_rhs_from_h0(h0b), _prep_rhs_from_h0(h0f[::-1])], axis=1)
    wih1f_t = _prep_w(inputs["w_ih1f"], 8)       # chunks 0-3 = h0f half
    wih1b_t = _prep_w(inputs["w_ih1b"], 8)
    wih1b_t = np.concatenate([wih1b_t[4:], wih1b_t[:4]], axis=0)
    fwd = {"rhs": rhs_f, "wih": wih1f_t, "biasm": _prep_biasq(inputs["b1f"]),
           "whh": _prep_whh(inputs["w_hh1f"]), "ident": _IDENT}
    bwd = {"rhs": rhs_b, "wih": wih1b_t, "biasm": _prep_biasq(inputs["b1b"]),
           "whh": _prep_whh(inputs["w_hh1b"]), "ident": _IDENT}
    return [fwd if c % 2 == 0 else bwd for c in range(NCORES)]


def assemble_out(h1f, h1b):
    out = np.concatenate(
        [_unpack_h(h1f, False), _unpack_h(h1b, True)], axis=2)
    return np.ascontiguousarray(out).astype(np.float32)


def kernel(x, w_ih0f, w_hh0f, b0f, w_ih0b, w_hh0b, b0b,
           w_ih1f, w_hh1f, b1f, w_ih1b, w_hh1b, b1b):
    inputs = dict(x=x, w_ih0f=w_ih0f, w_hh0f=w_hh0f, b0f=b0f,
                  w_ih0b=w_ih0b, w_hh0b=w_hh0b, b0b=b0b,
                  w_ih1f=w_ih1f, w_hh1f=w_hh1f, b1f=b1f,
                  w_ih1b=w_ih1b, w_hh1b=w_hh1b, b1b=b1b)

    # ---- launch 1: layer 0 (xg prologue + recurrence) ----
    res = _run(_get_nc(4), build_maps_l0(inputs))
    h0f = res.results[0]["hout"]
    h0b = res.results[1]["hout"]

    # ---- launch 2: layer 1 ----
    res = _run(_get_nc(8), build_maps_l1(inputs, h0f, h0b))
    return assemble_out(res.results[0]["hout"], res.results[1]["hout"])
